# revision 1
# baseline (speedup 1.0000x reference)
"""Trainium2 Bass kernel for batched tanh-attention flat-softmax.

Per batch b:
    Q = query[b] @ W_query; K = query[b] @ W_key      # [S, 64]
    s = tanh(Q @ K.T) * 10                            # [S, S]
    s[diag] = -inf
    out[b] = softmax(s.flatten())

Sharding: data-parallel over batch across 8 NeuronCores (6 batches/core),
W_query/W_key replicated; no cross-core communication.

Numerics: tanh(x)*10 is bounded in [-10,10], so softmax needs no max
subtraction: out = exp(10*tanh(s)) / sum(...). The diagonal is clamped to
-1e4 on the tanh output, so exp underflows to exactly 0 (matching the
reference's additive -1e8 mask).

Precision strategy: all matmuls run in bf16 hi/lo split form (1 cyc/row on
PE vs 4 for fp32) with fp32 PSUM accumulation:
  - query is split once: q = qh + ql (bf16 pair, exact to ~2^-17)
  - queryT is built by hardware DMA-transpose of the bf16 halves (free)
  - projections: [Q;K] = [Wq|Wk]h.T qh + [Wq|Wk]h.T ql + [Wq|Wk]l.T qh
  - scores: [Qh;Ql].T [Kh;Kh] + Qh.T Kl  (packed into 128 partitions)
Dropped terms are O(2^-18) relative; measured end-to-end max elementwise
error vs the fp32 reference is ~2e-3 on tiny elements, L2 ~1e-5.
"""

import numpy as np

import concourse.bass as bass
import concourse.bass_isa as bass_isa
import concourse.mybir as mybir
import concourse.tile as tile
from concourse import bacc
from concourse.bass_utils import run_bass_kernel_spmd

B = 48
S = 1024
D = 128
DK = 64
N_CORES = 8
BPC = B // N_CORES
P = 128
NQ = S // P
F32 = mybir.dt.float32
BF16 = mybir.dt.bfloat16
AL = mybir.AluOpType

TANH_CLIP = 10.0
DIAG_NEG = -1.0e4


def build_bass() -> bass.Bass:
    nc = bacc.Bacc(None, target_bir_lowering=False)

    q_d = nc.dram_tensor("query", [BPC, S, D], F32, kind="ExternalInput")
    wq_d = nc.dram_tensor("W_query", [D, DK], F32, kind="ExternalInput")
    wk_d = nc.dram_tensor("W_key", [D, DK], F32, kind="ExternalInput")
    out_d = nc.dram_tensor("out", [BPC, S, S], F32, kind="ExternalOutput")

    with tile.TileContext(nc) as tc:
        with (
            tc.tile_pool(name="singles", bufs=1) as singles,
            tc.tile_pool(name="qload", bufs=2) as qload,
            tc.tile_pool(name="qtp", bufs=2) as qtp,
            tc.tile_pool(name="projsb", bufs=2) as projsb,
            tc.tile_pool(name="tbuf", bufs=3) as tbuf,
            tc.tile_pool(name="small", bufs=2) as small,
            tc.tile_pool(name="dram", bufs=2, space="DRAM") as dpool,
            tc.tile_pool(name="ps_sc", bufs=4, space="PSUM") as ps_sc,
        ):
            # --- one-time setup ---
            # diag clamp mask: min(t, dmask) forces diagonal to -1e4
            dmask = singles.tile([P, P], F32)
            nc.vector.memset(dmask, 3.0e38)
            nc.gpsimd.affine_select(
                out=dmask,
                in_=dmask,
                compare_op=AL.not_equal,
                fill=DIAG_NEG,
                base=0,
                pattern=[[-1, P]],
                channel_multiplier=1,
            )

            # W stacked [Wq | Wk] as fp32, then bf16 hi/lo
            w32 = singles.tile([D, 2 * DK], F32)
            nc.sync.dma_start(w32[:, 0:DK], wq_d[:, :])
            nc.sync.dma_start(w32[:, DK:2 * DK], wk_d[:, :])
            wh = singles.tile([D, 2 * DK], BF16)
            nc.vector.tensor_copy(wh, w32)
            wl = singles.tile([D, 2 * DK], BF16)
            nc.vector.tensor_tensor(wl, w32, wh, AL.subtract)

            # ---- software-pipelined batch loop --------------------------
            # Emission order is engine-aware so no engine's in-order queue
            # blocks another batch's ready work (esp. DVE: batch i+1's
            # operand prep must precede batch i's normalize).

            def load_and_transpose(b):
                """Load query[b], split bf16 hi/lo, DMA-transpose both.

                Plain copy DMAs go through SWDGE (gpsimd) so the Sync
                HWDGE ring only ever runs DMA_TRANSPOSE — avoids the
                xbar-mode-switch serialization between transfer kinds."""
                q_sb = qload.tile([P, NQ, D], F32, tag="q")
                nc.sync.dma_start(
                    q_sb, q_d[b].rearrange("(n p) d -> p n d", p=P)
                )
                qh_sb = qload.tile([P, NQ, D], BF16, tag="qh")
                nc.vector.tensor_copy(qh_sb, q_sb)
                ql_sb = qload.tile([P, NQ, D], BF16, tag="ql")
                nc.vector.tensor_tensor(ql_sb, q_sb, qh_sb, AL.subtract)

                qhT = qtp.tile([D, S], BF16, tag="qhT")
                qlT = qtp.tile([D, S], BF16, tag="qlT")
                for src, dst, tg in ((qh_sb, qhT, "h"), (ql_sb, qlT, "l")):
                    scratch = dpool.tile([S, D], BF16, tag="scr" + tg)
                    nc.sync.dma_start(
                        scratch.rearrange("(n p) d -> p n d", p=P), src
                    )
                    nc.sync.dma_start_transpose(dst, scratch)
                return qhT, qlT

            def proj(qhT, qlT):
                """[Q; K] = [Wq|Wk].T queryT in bf16 hi/lo, fp32 psum."""
                pp = ps_sc.tile([P, S], F32, tag="sc", name="pp")
                for h in range(2):
                    cols = slice(h * 512, (h + 1) * 512)
                    nc.tensor.matmul(
                        pp[:, cols], wh, qhT[:, cols], start=True, stop=False
                    )
                    nc.tensor.matmul(
                        pp[:, cols], wh, qlT[:, cols], start=False, stop=False
                    )
                    nc.tensor.matmul(
                        pp[:, cols], wl, qhT[:, cols], start=False, stop=True
                    )
                return pp

            def build_stacks(pp):
                """Split Q/K psum into bf16 hi/lo matmul operands."""
                hb = projsb.tile([P, S], BF16, tag="hb")   # [Qh; Kh]
                nc.vector.tensor_copy(hb, pp)
                lb = projsb.tile([P, S], BF16, tag="lb")   # [Ql; Kl]
                nc.vector.tensor_tensor(lb, pp, hb, AL.subtract)

                qstack = projsb.tile([P, S], BF16, tag="qstack")  # [Qh; Ql]
                nc.vector.tensor_copy(qstack[0:DK], hb[0:DK])
                nc.vector.tensor_copy(qstack[DK:P], lb[0:DK])
                khh = projsb.tile([P, S], BF16, tag="khh")        # [Kh; Kh]
                nc.vector.tensor_copy(khh[0:DK], hb[DK:P])
                nc.vector.tensor_copy(khh[DK:P], hb[DK:P])
                # duplicated stacks for row-group-packed correction matmuls
                qhh = projsb.tile([P, S], BF16, tag="qhh")        # [Qh; Qh]
                nc.vector.tensor_copy(qhh[0:DK], hb[0:DK])
                nc.vector.tensor_copy(qhh[DK:P], hb[0:DK])
                kll = projsb.tile([P, S], BF16, tag="kll")        # [Kl; Kl]
                nc.vector.tensor_copy(kll[0:DK], lb[DK:P])
                nc.vector.tensor_copy(kll[DK:P], lb[DK:P])
                return qstack, khh, qhh, kll

            def scores_pair(t_sb, qstack, khh, qhh, kll, j):
                """Two 128-row score chunks (qc=2j, 2j+1), one 2-bank PSUM
                tile each. Main matmuls are 128-contraction; the two 64-
                contraction Qh.T Kl corrections run CONCURRENTLY in
                different PE row groups via tile_position."""
                qc0, qc1 = 2 * j, 2 * j + 1
                sc0 = ps_sc.tile([P, S], F32, tag="sc", name="sc0")
                sc1 = ps_sc.tile([P, S], F32, tag="sc", name="sc1")
                sl0 = slice(qc0 * P, (qc0 + 1) * P)
                sl1 = slice(qc1 * P, (qc1 + 1) * P)
                for h in range(2):
                    cols = slice(h * 512, (h + 1) * 512)
                    nc.tensor.matmul(
                        sc0[:, cols], qstack[:, sl0], khh[:, cols],
                        start=True, stop=False,
                    )
                    nc.tensor.matmul(
                        sc1[:, cols], qstack[:, sl1], khh[:, cols],
                        start=True, stop=False,
                    )
                    nc.tensor.matmul(
                        sc0[:, cols], qhh[0:DK, sl0], kll[0:DK, cols],
                        start=False, stop=True, tile_position=(0, 0),
                    )
                    nc.tensor.matmul(
                        sc1[:, cols], qhh[DK:P, sl1], kll[DK:P, cols],
                        start=False, stop=True, tile_position=(DK, 0),
                    )
                nc.scalar.activation(
                    out=t_sb[:, qc0],
                    in_=sc0,
                    func=mybir.ActivationFunctionType.Tanh,
                )
                nc.scalar.activation(
                    out=t_sb[:, qc1],
                    in_=sc1,
                    func=mybir.ActivationFunctionType.Tanh,
                )
                # clamp both diagonal blocks with one strided DVE op;
                # block qc sits at free offset qc*(S+P) with length P
                blk0 = t_sb[:, qc0, qc0 * P:(qc0 + 1) * P]
                diag_ap = bass.AP(
                    tensor=blk0.tensor,
                    offset=blk0.offset,
                    ap=[blk0.ap[0], [S + P, 2], [1, P]],
                )
                m0 = dmask[:, 0:P]
                mask_ap = bass.AP(
                    tensor=m0.tensor,
                    offset=m0.offset,
                    ap=[m0.ap[0], [0, 2], [1, P]],
                )
                nc.vector.tensor_tensor(diag_ap, diag_ap, mask_ap, AL.min)

            def exp_half(t_sb, rs, hidx):
                """exp(10*t) in place over half the batch rows, row sums
                accumulated into rs[:, hidx]."""
                nc.scalar.activation(
                    out=t_sb[:, 4 * hidx:4 * hidx + 4],
                    in_=t_sb[:, 4 * hidx:4 * hidx + 4],
                    func=mybir.ActivationFunctionType.Exp,
                    scale=TANH_CLIP,
                    accum_out=rs[:, hidx:hidx + 1],
                )

            def finish_batch(rs):
                """Z from the two half-sums; rz = 1/Z on all partitions."""
                zall = small.tile([P, 2], F32, tag="zall")
                nc.gpsimd.partition_all_reduce(
                    zall, rs, channels=P, reduce_op=bass_isa.ReduceOp.add
                )
                zsum = small.tile([P, 1], F32, tag="zsum")
                nc.vector.tensor_tensor(
                    zsum, zall[:, 0:1], zall[:, 1:2], AL.add
                )
                rz = small.tile([P, 1], F32, tag="rz")
                nc.vector.reciprocal(rz, zsum)
                return rz

            def store_batch(b, t_sb, rz):
                nc.vector.tensor_scalar_mul(t_sb, t_sb, rz)
                # big store via SWDGE on the (otherwise idle) GpSimd queue:
                # its wait-for-normalize must not block the SP ring's loads
                # and transposes, nor any compute engine's queue
                nc.gpsimd.dma_start(
                    out_d[b].rearrange("(n p) s -> p n s", p=P), t_sb
                )

            # prologue
            qhT, qlT = load_and_transpose(0)
            pp = proj(qhT, qlT)
            ops = build_stacks(pp)
            pending = None  # (b, t_sb, rz) awaiting normalize+store

            for b in range(BPC):
                t_sb = tbuf.tile([P, NQ, S], F32, tag="t")
                rs = small.tile([P, 2], F32, tag="rs")

                if pending is not None:
                    store_batch(*pending)
                    pending = None
                if b + 1 < BPC:
                    # start next batch's load/split/transpose chain early;
                    # it needs ~10us of DMA latency to land
                    nqhT, nqlT = load_and_transpose(b + 1)

                scores_pair(t_sb, *ops, 0)
                scores_pair(t_sb, *ops, 1)
                exp_half(t_sb, rs, 0)
                scores_pair(t_sb, *ops, 2)

                if b + 1 < BPC:
                    npp = proj(nqhT, nqlT)
                    nops = build_stacks(npp)

                scores_pair(t_sb, *ops, 3)
                if b + 1 < BPC:
                    ops = nops

                exp_half(t_sb, rs, 1)
                rz = finish_batch(rs)
                pending = (b, t_sb, rz)

            store_batch(*pending)

    nc.compile()
    return nc


_CACHED_NC = None


def kernel(**inputs: np.ndarray) -> np.ndarray:
    global _CACHED_NC
    query = np.ascontiguousarray(np.asarray(inputs["query"], dtype=np.float32))
    wq = np.ascontiguousarray(np.asarray(inputs["W_query"], dtype=np.float32))
    wk = np.ascontiguousarray(np.asarray(inputs["W_key"], dtype=np.float32))
    assert query.shape == (B, S, D), query.shape

    if _CACHED_NC is None:
        _CACHED_NC = build_bass()
    nc = _CACHED_NC

    in_maps = [
        {
            "query": query[c * BPC:(c + 1) * BPC],
            "W_query": wq,
            "W_key": wk,
        }
        for c in range(N_CORES)
    ]
    res = run_bass_kernel_spmd(nc, in_maps, core_ids=list(range(N_CORES)))
    out = np.concatenate(
        [r["out"].reshape(BPC, S * S) for r in res.results], axis=0
    )
    return out



# revision 2
# speedup vs baseline: 1.2674x; 1.2674x over previous
"""Trainium2 Bass kernel for batched tanh-attention flat-softmax.

Per batch b:
    Q = query[b] @ W_query; K = query[b] @ W_key      # [S, 64]
    s = tanh(Q @ K.T) * 10                            # [S, S]
    s[diag] = -inf
    out[b] = softmax(s.flatten())

Sharding: data-parallel over batch across 8 NeuronCores (6 batches/core),
W_query/W_key replicated; no cross-core communication.

Numerics: tanh(x)*10 is bounded in [-10,10], so softmax needs no max
subtraction: out = exp(10*tanh(s)) / sum(...). The diagonal is clamped to
-30 on the raw scores (min on PSUM), so exp(10*tanh(-30)) = e^-10, which
is ~4e-15 of the total mass -- indistinguishable from the reference's 0.

Precision strategy (validated vs fp64 reference: rel L2 ~6.3e-3, gate 2e-2):
  - query is cast to bf16 (RNE) on the host; the kernel DMA-transposes it
    straight from DRAM (2-byte xbar transpose), so there is no fp32 load,
    no hi/lo split pass, and no DRAM scratch roundtrip.
  - projections keep a hi/lo split for W only (wh + wl, both bf16):
    [Q;K] = wh.T qT + wl.T qT in fp32 PSUM.
  - scores use a single bf16 Qh.T Kh matmul (64-contraction). Two row
    chunks run CONCURRENTLY in separate PE row groups via tile_position,
    fed by two projection layouts A=[Q;K], B=[K;Q] so both tile rows read
    stationary/moving operands from their own partition range with zero
    shuffle copies.
  - tanh output is stored fp16, exp runs in-place fp16 (accum fp32).
"""

import numpy as np
import ml_dtypes

import concourse.bass as bass
import concourse.bass_isa as bass_isa
import concourse.mybir as mybir
import concourse.tile as tile
from concourse import bacc
from concourse.bass_utils import run_bass_kernel_spmd

B = 48
S = 1024
D = 128
DK = 64
N_CORES = 8
BPC = B // N_CORES
P = 128
NQ = S // P
F32 = mybir.dt.float32
F16 = mybir.dt.float16
BF16 = mybir.dt.bfloat16
AL = mybir.AluOpType

TANH_CLIP = 10.0
DIAG_NEG = -30.0


def build_bass() -> bass.Bass:
    nc = bacc.Bacc(None, target_bir_lowering=False)

    qh_d = nc.dram_tensor("query", [BPC, S, D], BF16, kind="ExternalInput")
    wq_d = nc.dram_tensor("W_query", [D, DK], F32, kind="ExternalInput")
    wk_d = nc.dram_tensor("W_key", [D, DK], F32, kind="ExternalInput")
    out_d = nc.dram_tensor("out", [BPC, S, S], F32, kind="ExternalOutput")

    with tile.TileContext(nc) as tc:
        with (
            tc.tile_pool(name="singles", bufs=1) as singles,
            tc.tile_pool(name="qtp", bufs=2) as qtp,
            tc.tile_pool(name="hbp", bufs=2) as hbp,
            tc.tile_pool(name="tbuf", bufs=2) as tbuf,
            tc.tile_pool(name="obuf", bufs=2) as obuf,
            tc.tile_pool(name="small", bufs=2) as small,
            tc.tile_pool(name="ps", bufs=2, space="PSUM") as psp,
        ):
            # --- one-time setup ---
            # diag clamp mask: min(s, dmask) forces diagonal to -30
            dmask = singles.tile([P, P], F32)
            nc.vector.memset(dmask, 3.0e38)
            nc.gpsimd.affine_select(
                out=dmask,
                in_=dmask,
                compare_op=AL.not_equal,
                fill=DIAG_NEG,
                base=0,
                pattern=[[-1, P]],
                channel_multiplier=1,
            )

            # weight stacks: A = [Wq | Wk], B = [Wk | Wq], each as bf16 hi/lo
            w32 = singles.tile([D, 4 * DK], F32)
            nc.sync.dma_start(w32[:, 0 * DK:1 * DK], wq_d[:, :])
            nc.sync.dma_start(w32[:, 1 * DK:2 * DK], wk_d[:, :])
            nc.sync.dma_start(w32[:, 2 * DK:3 * DK], wk_d[:, :])
            nc.sync.dma_start(w32[:, 3 * DK:4 * DK], wq_d[:, :])
            wh = singles.tile([D, 4 * DK], BF16)
            nc.vector.tensor_copy(wh, w32)
            wl = singles.tile([D, 4 * DK], BF16)
            nc.vector.tensor_tensor(wl, w32, wh, AL.subtract)
            whA, whB = wh[:, 0:P], wh[:, P:2 * P]
            wlA, wlB = wl[:, 0:P], wl[:, P:2 * P]

            def load_q(b):
                """DMA-transpose query[b] (bf16) straight from DRAM."""
                qhT = qtp.tile([D, S], BF16, tag="qhT")
                nc.sync.dma_start_transpose(qhT, qh_d[b])
                return qhT

            def proj(qhT):
                """pp[:,0] = A = [Q;K], pp[:,1] = B = [K;Q] (fp32 psum)."""
                pp = psp.tile([P, 2, S], F32, tag="ps", name="pp")
                terms = (
                    (whA, 0, True, False),
                    (whB, 1, True, False),
                    (wlA, 0, False, True),
                    (wlB, 1, False, True),
                )
                for w, half, st, sp in terms:
                    for h in range(2):
                        cols = slice(h * 512, (h + 1) * 512)
                        nc.tensor.matmul(
                            pp[:, half, cols], w, qhT[:, cols],
                            start=st, stop=sp,
                        )
                return pp

            def cast_hb(pp):
                hb = hbp.tile([P, 2, S], BF16, tag="hb")
                nc.vector.tensor_copy(hb, pp)
                return hb

            def scores_pair(t_sb, hb, j):
                """Two 128-row score chunks (qc=2j, 2j+1) in one 4-bank PSUM
                tile; the two 64-contraction matmuls stream CONCURRENTLY in
                different PE row groups. One strided diag-min, one tanh."""
                sc = psp.tile([P, 2, S], F32, tag="ps", name=f"sc{j}")
                sl0 = slice((2 * j) * P, (2 * j + 1) * P)
                sl1 = slice((2 * j + 1) * P, (2 * j + 2) * P)
                A, Bv = hb[:, 0], hb[:, 1]
                for h in range(2):
                    cols = slice(h * 512, (h + 1) * 512)
                    nc.tensor.matmul(
                        sc[:, 0, cols], A[0:DK, sl0], Bv[0:DK, cols],
                        start=True, stop=True, tile_position=(0, 0),
                    )
                    nc.tensor.matmul(
                        sc[:, 1, cols], Bv[DK:P, sl1], A[DK:P, cols],
                        start=True, stop=True, tile_position=(DK, 0),
                    )
                # clamp both diagonal blocks with one strided DVE min on PSUM;
                # chunk qc0's block is at free offset 2j*P, qc1's is S+P later
                blk0 = sc[:, 0, (2 * j) * P:(2 * j + 1) * P]
                diag_ap = bass.AP(
                    tensor=blk0.tensor,
                    offset=blk0.offset,
                    ap=[blk0.ap[0], [S + P, 2], [1, P]],
                )
                m0 = dmask[:, 0:P]
                mask_ap = bass.AP(
                    tensor=m0.tensor,
                    offset=m0.offset,
                    ap=[m0.ap[0], [0, 2], [1, P]],
                )
                nc.vector.tensor_tensor(diag_ap, diag_ap, mask_ap, AL.min)
                nc.scalar.activation(
                    out=t_sb[:, 2 * j:2 * j + 2],
                    in_=sc,
                    func=mybir.ActivationFunctionType.Tanh,
                )

            def exp_half(t_sb, rs, hidx):
                """exp(10*t) in place (fp16) over half the rows, fp32 sums."""
                nc.scalar.activation(
                    out=t_sb[:, 4 * hidx:4 * hidx + 4],
                    in_=t_sb[:, 4 * hidx:4 * hidx + 4],
                    func=mybir.ActivationFunctionType.Exp,
                    scale=TANH_CLIP,
                    accum_out=rs[:, hidx:hidx + 1],
                )

            def finish_batch(rs):
                zall = small.tile([P, 2], F32, tag="zall")
                nc.gpsimd.partition_all_reduce(
                    zall, rs, channels=P, reduce_op=bass_isa.ReduceOp.add
                )
                zsum = small.tile([P, 1], F32, tag="zsum")
                nc.vector.tensor_tensor(
                    zsum, zall[:, 0:1], zall[:, 1:2], AL.add
                )
                rz = small.tile([P, 1], F32, tag="rz")
                nc.vector.reciprocal(rz, zsum)
                return rz

            def norm_store_quarter(b, t_sb, o_sb, rz, qtr):
                """Normalize (fp16 -> fp32) + store one quarter of batch b."""
                sl = slice(2 * qtr, 2 * qtr + 2)
                nc.vector.tensor_scalar_mul(o_sb[:, sl], t_sb[:, sl], rz)
                nc.gpsimd.dma_start(
                    out_d[b].rearrange("(n p) s -> p n s", p=P)[:, sl],
                    o_sb[:, sl],
                )

            # ---- software-pipelined batch loop --------------------------
            qhT = load_q(0)
            hb = cast_hb(proj(qhT))
            pending = None  # (b, t_sb, o_sb, rz) awaiting normalize+store

            for b in range(BPC):
                t_sb = tbuf.tile([P, NQ, S], F16, tag="t")
                o_sb = obuf.tile([P, NQ, S], F32, tag="o")
                rs = small.tile([P, 2], F32, tag="rs")

                if b + 1 < BPC:
                    nqhT = load_q(b + 1)

                scores_pair(t_sb, hb, 0)
                if pending is not None:
                    norm_store_quarter(*pending, 0)
                scores_pair(t_sb, hb, 1)
                if pending is not None:
                    norm_store_quarter(*pending, 1)
                exp_half(t_sb, rs, 0)
                scores_pair(t_sb, hb, 2)
                if pending is not None:
                    norm_store_quarter(*pending, 2)
                if b + 1 < BPC:
                    nhb = cast_hb(proj(nqhT))
                scores_pair(t_sb, hb, 3)
                if pending is not None:
                    norm_store_quarter(*pending, 3)
                if b + 1 < BPC:
                    hb = nhb
                exp_half(t_sb, rs, 1)
                rz = finish_batch(rs)
                pending = (b, t_sb, o_sb, rz)

            for qtr in range(4):
                norm_store_quarter(*pending, qtr)

    nc.compile()
    return nc


_CACHED_NC = None


def kernel(**inputs: np.ndarray) -> np.ndarray:
    global _CACHED_NC
    query = np.asarray(inputs["query"], dtype=np.float32)
    wq = np.ascontiguousarray(np.asarray(inputs["W_query"], dtype=np.float32))
    wk = np.ascontiguousarray(np.asarray(inputs["W_key"], dtype=np.float32))
    assert query.shape == (B, S, D), query.shape
    qh = np.ascontiguousarray(query.astype(ml_dtypes.bfloat16))

    if _CACHED_NC is None:
        _CACHED_NC = build_bass()
    nc = _CACHED_NC

    in_maps = [
        {
            "query": qh[c * BPC:(c + 1) * BPC],
            "W_query": wq,
            "W_key": wk,
        }
        for c in range(N_CORES)
    ]
    res = run_bass_kernel_spmd(nc, in_maps, core_ids=list(range(N_CORES)))
    out = np.concatenate(
        [r["out"].reshape(BPC, S * S) for r in res.results], axis=0
    )
    return out


# revision 4
# speedup vs baseline: 1.2923x; 1.0196x over previous
"""Trainium2 Bass kernel for batched tanh-attention flat-softmax.

Per batch b:
    Q = query[b] @ W_query; K = query[b] @ W_key      # [S, 64]
    s = tanh(Q @ K.T) * 10                            # [S, S]
    s[diag] = -inf
    out[b] = softmax(s.flatten())

Sharding: data-parallel over batch across 8 NeuronCores (6 batches/core),
W_query/W_key replicated; no cross-core communication.

Numerics: tanh(x)*10 is bounded in [-10,10], so softmax needs no max
subtraction: out = exp(10*tanh(s)) / sum(...). The diagonal is clamped to
-30 on the raw scores (min on PSUM), so exp(10*tanh(-30)) = e^-10, which
is ~4e-15 of the total mass -- indistinguishable from the reference's 0.

Precision strategy (validated vs fp64 reference: rel L2 ~6.3e-3, gate 2e-2):
  - query is cast to bf16 (RNE) on the host; the kernel DMA-transposes it
    straight from DRAM (2-byte xbar transpose), so there is no fp32 load,
    no hi/lo split pass, and no DRAM scratch roundtrip.
  - projections keep a hi/lo split for W only (wh + wl, both bf16):
    [Q;K] = wh.T qT + wl.T qT in fp32 PSUM.
  - scores use a single bf16 Qh.T Kh matmul (64-contraction). Two row
    chunks run CONCURRENTLY in separate PE row groups via tile_position,
    fed by two projection layouts A=[Q;K], B=[K;Q] so both tile rows read
    stationary/moving operands from their own partition range with zero
    shuffle copies.
  - tanh output is stored fp16, exp runs in-place fp16 (accum fp32).
"""

import numpy as np
import ml_dtypes

import concourse.bass as bass
import concourse.bass_isa as bass_isa
import concourse.mybir as mybir
import concourse.tile as tile
from concourse import bacc
from concourse.bass_utils import run_bass_kernel_spmd

B = 48
S = 1024
D = 128
DK = 64
N_CORES = 8
BPC = B // N_CORES
P = 128
NQ = S // P
F32 = mybir.dt.float32
F16 = mybir.dt.float16
BF16 = mybir.dt.bfloat16
AL = mybir.AluOpType

TANH_CLIP = 10.0
DIAG_NEG = -30.0


def build_bass() -> bass.Bass:
    nc = bacc.Bacc(None, target_bir_lowering=False)

    qh_d = nc.dram_tensor("query", [BPC, S, D], BF16, kind="ExternalInput")
    wq_d = nc.dram_tensor("W_query", [D, DK], F32, kind="ExternalInput")
    wk_d = nc.dram_tensor("W_key", [D, DK], F32, kind="ExternalInput")
    out_d = nc.dram_tensor("out", [BPC, S, S], F32, kind="ExternalOutput")

    with tile.TileContext(nc) as tc:
        with (
            tc.tile_pool(name="singles", bufs=1) as singles,
            tc.tile_pool(name="qtp", bufs=2) as qtp,
            tc.tile_pool(name="hbp", bufs=2) as hbp,
            tc.tile_pool(name="tbuf", bufs=2) as tbuf,
            tc.tile_pool(name="obuf", bufs=2) as obuf,
            tc.tile_pool(name="small", bufs=2) as small,
            tc.tile_pool(name="ps", bufs=2, space="PSUM") as psp,
        ):
            # --- one-time setup ---
            # first batch's transpose goes out on the sync ring before the
            # weight loads: its data is on the critical path to first tanh
            qhT0 = qtp.tile([D, S], BF16, tag="qhT")
            nc.sync.dma_start_transpose(qhT0, qh_d[0])

            # diag clamp mask: min(s, dmask) forces diagonal to -30
            dmask = singles.tile([P, P], F32)
            nc.vector.memset(dmask, 3.0e38)
            nc.gpsimd.affine_select(
                out=dmask,
                in_=dmask,
                compare_op=AL.not_equal,
                fill=DIAG_NEG,
                base=0,
                pattern=[[-1, P]],
                channel_multiplier=1,
            )

            # weight stacks: A = [Wq | Wk], B = [Wk | Wq], each as bf16 hi/lo
            w32 = singles.tile([D, 4 * DK], F32)
            nc.sync.dma_start(w32[:, 0 * DK:1 * DK], wq_d[:, :])
            nc.sync.dma_start(w32[:, 1 * DK:2 * DK], wk_d[:, :])
            nc.sync.dma_start(w32[:, 2 * DK:3 * DK], wk_d[:, :])
            nc.sync.dma_start(w32[:, 3 * DK:4 * DK], wq_d[:, :])
            wh = singles.tile([D, 4 * DK], BF16)
            nc.vector.tensor_copy(wh, w32)
            wl = singles.tile([D, 4 * DK], BF16)
            nc.vector.tensor_tensor(wl, w32, wh, AL.subtract)
            whA, whB = wh[:, 0:P], wh[:, P:2 * P]
            wlA, wlB = wl[:, 0:P], wl[:, P:2 * P]

            def load_q(b):
                """DMA-transpose query[b] (bf16) straight from DRAM."""
                qhT = qtp.tile([D, S], BF16, tag="qhT")
                nc.sync.dma_start_transpose(qhT, qh_d[b])
                return qhT

            def proj(qhT):
                """pp[:,0] = A = [Q;K], pp[:,1] = B = [K;Q] (fp32 psum)."""
                pp = psp.tile([P, 2, S], F32, tag="ps", name="pp")
                terms = (
                    (whA, 0, True, False),
                    (whB, 1, True, False),
                    (wlA, 0, False, True),
                    (wlB, 1, False, True),
                )
                for w, half, st, sp in terms:
                    for h in range(2):
                        cols = slice(h * 512, (h + 1) * 512)
                        nc.tensor.matmul(
                            pp[:, half, cols], w, qhT[:, cols],
                            start=st, stop=sp,
                        )
                return pp

            def cast_hb(pp):
                hb = hbp.tile([P, 2, S], BF16, tag="hb")
                nc.vector.tensor_copy(hb, pp)
                return hb

            def scores_pair(t_sb, hb, j):
                """Two 128-row score chunks (qc=2j, 2j+1) in one 4-bank PSUM
                tile; the two 64-contraction matmuls stream CONCURRENTLY in
                different PE row groups. One strided diag-min, one tanh."""
                sc = psp.tile([P, 2, S], F32, tag="ps", name=f"sc{j}")
                sl0 = slice((2 * j) * P, (2 * j + 1) * P)
                sl1 = slice((2 * j + 1) * P, (2 * j + 2) * P)
                A, Bv = hb[:, 0], hb[:, 1]
                for h in range(2):
                    cols = slice(h * 512, (h + 1) * 512)
                    nc.tensor.matmul(
                        sc[:, 0, cols], A[0:DK, sl0], Bv[0:DK, cols],
                        start=True, stop=True, tile_position=(0, 0),
                    )
                    nc.tensor.matmul(
                        sc[:, 1, cols], Bv[DK:P, sl1], A[DK:P, cols],
                        start=True, stop=True, tile_position=(DK, 0),
                    )
                # clamp both diagonal blocks with one strided DVE min on PSUM;
                # chunk qc0's block is at free offset 2j*P, qc1's is S+P later
                blk0 = sc[:, 0, (2 * j) * P:(2 * j + 1) * P]
                diag_ap = bass.AP(
                    tensor=blk0.tensor,
                    offset=blk0.offset,
                    ap=[blk0.ap[0], [S + P, 2], [1, P]],
                )
                m0 = dmask[:, 0:P]
                mask_ap = bass.AP(
                    tensor=m0.tensor,
                    offset=m0.offset,
                    ap=[m0.ap[0], [0, 2], [1, P]],
                )
                nc.vector.tensor_tensor(diag_ap, diag_ap, mask_ap, AL.min)
                nc.scalar.activation(
                    out=t_sb[:, 2 * j:2 * j + 2],
                    in_=sc,
                    func=mybir.ActivationFunctionType.Tanh,
                )

            def exp_half(t_sb, rs, hidx):
                """exp(10*t) in place (fp16) over half the rows, fp32 sums."""
                nc.scalar.activation(
                    out=t_sb[:, 4 * hidx:4 * hidx + 4],
                    in_=t_sb[:, 4 * hidx:4 * hidx + 4],
                    func=mybir.ActivationFunctionType.Exp,
                    scale=TANH_CLIP,
                    accum_out=rs[:, hidx:hidx + 1],
                )

            def all_reduce_z(rs):
                zall = small.tile([P, 2], F32, tag="zall")
                nc.gpsimd.partition_all_reduce(
                    zall, rs, channels=P, reduce_op=bass_isa.ReduceOp.add
                )
                return zall

            def finish_batch(zall):
                """zsum + reciprocal on DVE; emitted in the NEXT iteration
                after pair0's diag so they never block it in the queue."""
                zsum = small.tile([P, 1], F32, tag="zsum")
                nc.vector.tensor_tensor(
                    zsum, zall[:, 0:1], zall[:, 1:2], AL.add
                )
                rz = small.tile([P, 1], F32, tag="rz")
                nc.vector.reciprocal(rz, zsum)
                return rz

            def norm_store(b, t_sb, o_sb, rz, sl):
                """Normalize (fp16 -> fp32) + store chunk range sl of batch b."""
                nc.vector.tensor_scalar_mul(o_sb[:, sl], t_sb[:, sl], rz)
                nc.gpsimd.dma_start(
                    out_d[b].rearrange("(n p) s -> p n s", p=P)[:, sl],
                    o_sb[:, sl],
                )

            # ---- software-pipelined batch loop --------------------------
            hb = cast_hb(proj(qhT0))
            pending = None  # (b, t_sb, o_sb, zall) awaiting normalize+store

            for b in range(BPC):
                t_sb = tbuf.tile([P, NQ, S], F16, tag="t")
                o_sb = obuf.tile([P, NQ, S], F32, tag="o")
                rs = small.tile([P, 2], F32, tag="rs")

                if b + 1 < BPC:
                    nqhT = load_q(b + 1)

                scores_pair(t_sb, hb, 0)
                if pending is not None:
                    rz = finish_batch(pending[3])
                    norm_store(*pending[:3], rz, slice(0, 2))
                scores_pair(t_sb, hb, 1)
                if pending is not None:
                    norm_store(*pending[:3], rz, slice(2, 4))
                exp_half(t_sb, rs, 0)
                scores_pair(t_sb, hb, 2)
                if pending is not None:
                    norm_store(*pending[:3], rz, slice(4, 6))
                if b + 1 < BPC:
                    nhb = cast_hb(proj(nqhT))
                scores_pair(t_sb, hb, 3)
                if pending is not None:
                    norm_store(*pending[:3], rz, slice(6, 8))
                if b + 1 < BPC:
                    hb = nhb
                exp_half(t_sb, rs, 1)
                pending = (b, t_sb, o_sb, all_reduce_z(rs))

            # epilogue: chunk-granular so the first store fires right after
            # rz and the 4MB flush pipelines at chunk granularity
            rz = finish_batch(pending[3])
            for c in range(NQ):
                norm_store(*pending[:3], rz, slice(c, c + 1))

    nc.compile()
    return nc


_CACHED_NC = None


def kernel(**inputs: np.ndarray) -> np.ndarray:
    global _CACHED_NC
    query = np.asarray(inputs["query"], dtype=np.float32)
    wq = np.ascontiguousarray(np.asarray(inputs["W_query"], dtype=np.float32))
    wk = np.ascontiguousarray(np.asarray(inputs["W_key"], dtype=np.float32))
    assert query.shape == (B, S, D), query.shape
    qh = np.ascontiguousarray(query.astype(ml_dtypes.bfloat16))

    if _CACHED_NC is None:
        _CACHED_NC = build_bass()
    nc = _CACHED_NC

    in_maps = [
        {
            "query": qh[c * BPC:(c + 1) * BPC],
            "W_query": wq,
            "W_key": wk,
        }
        for c in range(N_CORES)
    ]
    res = run_bass_kernel_spmd(nc, in_maps, core_ids=list(range(N_CORES)))
    out = np.concatenate(
        [r["out"].reshape(BPC, S * S) for r in res.results], axis=0
    )
    return out


# revision 7
# speedup vs baseline: 1.3092x; 1.0131x over previous
"""Trainium2 Bass kernel for batched tanh-attention flat-softmax.

Per batch b:
    Q = query[b] @ W_query; K = query[b] @ W_key      # [S, 64]
    s = tanh(Q @ K.T) * 10                            # [S, S]
    s[diag] = -inf
    out[b] = softmax(s.flatten())

Sharding: data-parallel over batch across 8 NeuronCores (6 batches/core),
W_query/W_key replicated; no cross-core communication.

Numerics: tanh(x)*10 is bounded in [-10,10], so softmax needs no max
subtraction: out = exp(10*tanh(s)) / sum(...). The diagonal is clamped to
-30 on the raw scores (min on PSUM), so exp(10*tanh(-30)) = e^-10, which
is ~4e-15 of the total mass -- indistinguishable from the reference's 0.

Precision strategy (validated vs fp64 reference: rel L2 ~6.3e-3, gate 2e-2):
  - query is cast to bf16 (RNE) on the host; the kernel DMA-transposes it
    straight from DRAM (2-byte xbar transpose), so there is no fp32 load,
    no hi/lo split pass, and no DRAM scratch roundtrip.
  - projections keep a hi/lo split for W only (wh + wl, both bf16):
    [Q;K] = wh.T qT + wl.T qT in fp32 PSUM.
  - scores use a single bf16 Qh.T Kh matmul (64-contraction). Two row
    chunks run CONCURRENTLY in separate PE row groups via tile_position,
    fed by two projection layouts A=[Q;K], B=[K;Q] so both tile rows read
    stationary/moving operands from their own partition range with zero
    shuffle copies.
  - tanh output is stored fp16, exp runs in-place fp16 (accum fp32).
"""

import numpy as np
import ml_dtypes

import concourse.bass as bass
import concourse.bass_isa as bass_isa
import concourse.mybir as mybir
import concourse.tile as tile
from concourse import bacc
from concourse.bass_utils import run_bass_kernel_spmd

B = 48
S = 1024
D = 128
DK = 64
N_CORES = 8
BPC = B // N_CORES
P = 128
NQ = S // P
F32 = mybir.dt.float32
F16 = mybir.dt.float16
BF16 = mybir.dt.bfloat16
AL = mybir.AluOpType

TANH_CLIP = 10.0
DIAG_NEG = -30.0


def build_bass() -> bass.Bass:
    nc = bacc.Bacc(None, target_bir_lowering=False)

    qh_d = nc.dram_tensor("query", [BPC, S, D], BF16, kind="ExternalInput")
    wq_d = nc.dram_tensor("W_query", [D, DK], F32, kind="ExternalInput")
    wk_d = nc.dram_tensor("W_key", [D, DK], F32, kind="ExternalInput")
    out_d = nc.dram_tensor("out", [BPC, S, S], F32, kind="ExternalOutput")

    with tile.TileContext(nc) as tc:
        with (
            tc.tile_pool(name="singles", bufs=1) as singles,
            tc.tile_pool(name="qtp", bufs=2) as qtp,
            tc.tile_pool(name="hbp", bufs=2) as hbp,
            tc.tile_pool(name="tbuf", bufs=3) as tbuf,
            tc.tile_pool(name="obuf", bufs=3) as obuf,
            tc.tile_pool(name="small", bufs=2) as small,
            tc.tile_pool(name="ps", bufs=2, space="PSUM") as psp,
        ):
            # --- one-time setup ---
            # first batch's transpose goes out on the sync ring before the
            # weight loads: its data is on the critical path to first tanh
            qhT0 = qtp.tile([D, S], BF16, tag="qhT")
            nc.sync.dma_start_transpose(qhT0, qh_d[0])

            # diag clamp mask: min(s, dmask) forces diagonal to -30
            dmask = singles.tile([P, P], F32)
            nc.vector.memset(dmask, 3.0e38)
            nc.gpsimd.affine_select(
                out=dmask,
                in_=dmask,
                compare_op=AL.not_equal,
                fill=DIAG_NEG,
                base=0,
                pattern=[[-1, P]],
                channel_multiplier=1,
            )

            # weight stacks: A = [Wq | Wk], B = [Wk | Wq], each as bf16 hi/lo
            w32 = singles.tile([D, 4 * DK], F32)
            nc.sync.dma_start(w32[:, 0 * DK:1 * DK], wq_d[:, :])
            nc.sync.dma_start(w32[:, 1 * DK:2 * DK], wk_d[:, :])
            nc.sync.dma_start(w32[:, 2 * DK:3 * DK], wk_d[:, :])
            nc.sync.dma_start(w32[:, 3 * DK:4 * DK], wq_d[:, :])
            wh = singles.tile([D, 4 * DK], BF16)
            nc.vector.tensor_copy(wh, w32)
            wl = singles.tile([D, 4 * DK], BF16)
            nc.vector.tensor_tensor(wl, w32, wh, AL.subtract)
            whA, whB = wh[:, 0:P], wh[:, P:2 * P]
            wlA, wlB = wl[:, 0:P], wl[:, P:2 * P]

            def load_q(b):
                """DMA-transpose query[b] (bf16) straight from DRAM."""
                qhT = qtp.tile([D, S], BF16, tag="qhT")
                nc.sync.dma_start_transpose(qhT, qh_d[b])
                return qhT

            def proj(qhT):
                """pp[:,0] = A = [Q;K], pp[:,1] = B = [K;Q] (fp32 psum).
                Column-half-major order so the cast (and the first scores
                matmuls) can start after half the projection."""
                pp = psp.tile([P, 2, S], F32, tag="ps", name="pp")
                terms = (
                    (whA, 0, True, False),
                    (whB, 1, True, False),
                    (wlA, 0, False, True),
                    (wlB, 1, False, True),
                )
                for h in range(2):
                    cols = slice(h * 512, (h + 1) * 512)
                    for w, half, st, sp in terms:
                        nc.tensor.matmul(
                            pp[:, half, cols], w, qhT[:, cols],
                            start=st, stop=sp,
                        )
                return pp

            def cast_hb(pp):
                hb = hbp.tile([P, 2, S], BF16, tag="hb")
                for h in range(2):
                    cols = slice(h * 512, (h + 1) * 512)
                    nc.vector.tensor_copy(hb[:, :, cols], pp[:, :, cols])
                return hb

            def scores_pair(t_sb, hb, j):
                """Two 128-row score chunks (qc=2j, 2j+1) in one 4-bank PSUM
                tile; the two 64-contraction matmuls stream CONCURRENTLY in
                different PE row groups. One strided diag-min, one tanh."""
                sc = psp.tile([P, 2, S], F32, tag="ps", name=f"sc{j}")
                sl0 = slice((2 * j) * P, (2 * j + 1) * P)
                sl1 = slice((2 * j + 1) * P, (2 * j + 2) * P)
                A, Bv = hb[:, 0], hb[:, 1]
                for h in range(2):
                    cols = slice(h * 512, (h + 1) * 512)
                    nc.tensor.matmul(
                        sc[:, 0, cols], A[0:DK, sl0], Bv[0:DK, cols],
                        start=True, stop=True, tile_position=(0, 0),
                    )
                    nc.tensor.matmul(
                        sc[:, 1, cols], Bv[DK:P, sl1], A[DK:P, cols],
                        start=True, stop=True, tile_position=(DK, 0),
                    )
                # clamp both diagonal blocks with one strided DVE min on PSUM;
                # chunk qc0's block is at free offset 2j*P, qc1's is S+P later
                blk0 = sc[:, 0, (2 * j) * P:(2 * j + 1) * P]
                diag_ap = bass.AP(
                    tensor=blk0.tensor,
                    offset=blk0.offset,
                    ap=[blk0.ap[0], [S + P, 2], [1, P]],
                )
                m0 = dmask[:, 0:P]
                mask_ap = bass.AP(
                    tensor=m0.tensor,
                    offset=m0.offset,
                    ap=[m0.ap[0], [0, 2], [1, P]],
                )
                nc.vector.tensor_tensor(diag_ap, diag_ap, mask_ap, AL.min)
                nc.scalar.activation(
                    out=t_sb[:, 2 * j:2 * j + 2],
                    in_=sc,
                    func=mybir.ActivationFunctionType.Tanh,
                )

            def exp_half(t_sb, rs, hidx):
                """exp(10*t) in place (fp16) over half the rows, fp32 sums."""
                nc.scalar.activation(
                    out=t_sb[:, 4 * hidx:4 * hidx + 4],
                    in_=t_sb[:, 4 * hidx:4 * hidx + 4],
                    func=mybir.ActivationFunctionType.Exp,
                    scale=TANH_CLIP,
                    accum_out=rs[:, hidx:hidx + 1],
                )

            def all_reduce_z(rs):
                zall = small.tile([P, 2], F32, tag="zall")
                nc.gpsimd.partition_all_reduce(
                    zall, rs, channels=P, reduce_op=bass_isa.ReduceOp.add
                )
                return zall

            def finish_batch(zall):
                """zsum + reciprocal on DVE; emitted in the NEXT iteration
                after pair0's diag so they never block it in the queue."""
                zsum = small.tile([P, 1], F32, tag="zsum")
                nc.vector.tensor_tensor(
                    zsum, zall[:, 0:1], zall[:, 1:2], AL.add
                )
                rz = small.tile([P, 1], F32, tag="rz")
                nc.vector.reciprocal(rz, zsum)
                return rz

            def norm_store(b, t_sb, o_sb, rz, sl, ring):
                """Normalize (fp16 -> fp32) + store chunk range sl of batch b.
                Stores alternate between the SWDGE (gpsimd) and HWDGE (sync)
                rings so neither ring's stream gates the tail flush."""
                nc.vector.tensor_scalar_mul(o_sb[:, sl], t_sb[:, sl], rz)
                eng = nc.gpsimd if ring == 0 else nc.sync
                eng.dma_start(
                    out_d[b].rearrange("(n p) s -> p n s", p=P)[:, sl],
                    o_sb[:, sl],
                )

            # ---- software-pipelined batch loop --------------------------
            # pend1: newest finished batch (finish + chunks 0:6 this iter)
            # pend2: older batch with only chunks 6:8 left (done at iter top,
            #        after pair0's diag, so no DVE op ever delays diag0)
            hb = cast_hb(proj(qhT0))
            pend1 = None  # (b, t_sb, o_sb, zall)
            pend2 = None  # (b, t_sb, o_sb, rz)

            for b in range(BPC):
                t_sb = tbuf.tile([P, NQ, S], F16, tag="t")
                o_sb = obuf.tile([P, NQ, S], F32, tag="o")
                rs = small.tile([P, 2], F32, tag="rs")

                if b + 1 < BPC:
                    nqhT = load_q(b + 1)

                scores_pair(t_sb, hb, 0)
                if pend2 is not None:
                    norm_store(*pend2, slice(6, 8), 0)
                    pend2 = None
                if pend1 is not None:
                    rz1 = finish_batch(pend1[3])
                scores_pair(t_sb, hb, 1)
                if pend1 is not None:
                    norm_store(*pend1[:3], rz1, slice(0, 2), 1)
                exp_half(t_sb, rs, 0)
                scores_pair(t_sb, hb, 2)
                if pend1 is not None:
                    norm_store(*pend1[:3], rz1, slice(2, 4), 0)
                if b + 1 < BPC:
                    nhb = cast_hb(proj(nqhT))
                scores_pair(t_sb, hb, 3)
                if pend1 is not None:
                    norm_store(*pend1[:3], rz1, slice(4, 6), 1)
                if b + 1 < BPC:
                    hb = nhb
                exp_half(t_sb, rs, 1)
                if pend1 is not None:
                    pend2 = (*pend1[:3], rz1)
                pend1 = (b, t_sb, o_sb, all_reduce_z(rs))

            # epilogue: chunk-granular, stores fanned across both rings
            if pend2 is not None:
                norm_store(*pend2, slice(6, 8), 0)
            rz = finish_batch(pend1[3])
            for c in range(NQ):
                norm_store(*pend1[:3], rz, slice(c, c + 1), c % 2)

    nc.compile()
    return nc


_CACHED_NC = None


def kernel(**inputs: np.ndarray) -> np.ndarray:
    global _CACHED_NC
    query = np.asarray(inputs["query"], dtype=np.float32)
    wq = np.ascontiguousarray(np.asarray(inputs["W_query"], dtype=np.float32))
    wk = np.ascontiguousarray(np.asarray(inputs["W_key"], dtype=np.float32))
    assert query.shape == (B, S, D), query.shape
    qh = np.ascontiguousarray(query.astype(ml_dtypes.bfloat16))

    if _CACHED_NC is None:
        _CACHED_NC = build_bass()
    nc = _CACHED_NC

    in_maps = [
        {
            "query": qh[c * BPC:(c + 1) * BPC],
            "W_query": wq,
            "W_key": wk,
        }
        for c in range(N_CORES)
    ]
    res = run_bass_kernel_spmd(nc, in_maps, core_ids=list(range(N_CORES)))
    out = np.concatenate(
        [r["out"].reshape(BPC, S * S) for r in res.results], axis=0
    )
    return out
